# revision 13
# baseline (speedup 1.0000x reference)
"""GCN layer kernel for Trainium2, 8 NeuronCores.

out = D^-1/2 (A + I) D^-1/2 (x @ W) + bias   with A built dense from edge_index
(scatter-set semantics => duplicate edges collapse, matching the reference).

Sharding: 1D node/row partition over 8 cores (hardcoded). Each core builds the
transposed adjacency slab A_T[j, i] = A[r0+i, j] for its 1024 rows directly in
SBUF as 64 bf16 tiles via gpsimd local_scatter (per-column index lists, host-
bucketed; self loops appended on device as an extra index column), row-sums the
slab on the PE (ones-vector matmul) for its nodes' degrees, computes
z = deg^-1/2 * (x_shard @ W), all-gathers z, then contracts
out_T[d, i] = sum_j z[j, d] * A_T[j, i] on the PE with fp32 PSUM accumulation,
scales rows by deg^-1/2 and adds bias. Host only shards/buckets inputs and
transposes/concats the outputs.
"""

import sys

for _p in ("/opt/trn_rl_repo", "/root/.axon_site/_ro/trn_rl_repo"):
    if _p not in sys.path:
        sys.path.append(_p)

import numpy as np

import concourse.bacc as bacc
import concourse.bass as bass
import concourse.mybir as mybir
import concourse.tile as tile
from concourse.masks import make_identity

# Problem shape (hardcoded per contract)
N = 8192
DIN = 128
DOUT = 128
P = 128
NCORES = 8
NSHARD = N // NCORES          # 1024 rows per core
JT = N // P                   # 64 contraction tiles
MAXC = 23                     # max bucketed edges per (core, column)
NIDX = MAXC + 1               # + device-appended self-loop column (even)

BF16 = mybir.dt.bfloat16
F32 = mybir.dt.float32
I32 = mybir.dt.int32
I16 = mybir.dt.int16

_COMPILED = {}


def build_nc(debug: bool = False):
    nc = bacc.Bacc("TRN2", target_bir_lowering=False, debug=debug,
                   enable_asserts=False, num_devices=NCORES)

    # I/O
    x_c = nc.dram_tensor("x_c", [NSHARD, DIN], F32, kind="ExternalInput")
    w = nc.dram_tensor("w", [DIN, DOUT], F32, kind="ExternalInput")
    bias_in = nc.dram_tensor("bias_in", [DOUT, 1], F32, kind="ExternalInput")
    idx_in = nc.dram_tensor("idx_in", [JT, P, MAXC], I16, kind="ExternalInput")
    rowbase = nc.dram_tensor("rowbase", [P, 1], I32, kind="ExternalInput")
    out_t = nc.dram_tensor("out_t", [DOUT, NSHARD], F32, kind="ExternalOutput")

    # Internal DRAM
    z_dram = nc.dram_tensor("z_dram", [NSHARD, DOUT], BF16)
    deg_dram = nc.dram_tensor("deg_dram", [NSHARD, 1], F32)
    zall_dram = nc.dram_tensor("zall_dram", [N, DOUT], BF16, addr_space="Shared")

    with tile.TileContext(nc) as tc:
        with (
            tc.tile_pool(name="const", bufs=1) as cpool,
            tc.tile_pool(name="canv", bufs=JT) as canvpool,
            tc.tile_pool(name="work", bufs=1) as wpool,
            tc.tile_pool(name="psA", bufs=2, space="PSUM") as psA,
            tc.tile_pool(name="psD", bufs=1, space="PSUM") as psD,
            tc.tile_pool(name="psO", bufs=1, space="PSUM") as psO,
        ):
            # ---------- constants / small loads ----------
            ones_data = cpool.tile([P, NIDX], BF16, tag="ones_data")
            nc.gpsimd.memset(ones_data[:, :], 1.0)
            ones_col = cpool.tile([P, 1], BF16, tag="ones_col")
            nc.gpsimd.memset(ones_col[:, :], 1.0)

            ident = cpool.tile([P, P], F32, tag="ident")
            make_identity(nc, ident[:, :])

            w_sb = cpool.tile([DIN, DOUT], F32, tag="w_sb")
            nc.sync.dma_start(out=w_sb[:, :], in_=w[:, :])
            w_bf = cpool.tile([DIN, DOUT], BF16, tag="w_bf")
            nc.vector.tensor_copy(out=w_bf[:, :], in_=w_sb[:, :])
            bias_sb = cpool.tile([DOUT, 1], F32, tag="bias_sb")
            nc.sync.dma_start(out=bias_sb[:, :], in_=bias_in[:, :])

            rb_sb = cpool.tile([P, 1], I32, tag="rb_sb")
            nc.sync.dma_start(out=rb_sb[:, :], in_=rowbase[:, :])

            xc_sb = cpool.tile([P, NSHARD // P, DIN], F32, tag="xc_sb")
            nc.sync.dma_start(
                out=xc_sb[:, :, :],
                in_=x_c.ap().rearrange("(t p) d -> p t d", p=P),
            )

            # edge index lists [128, JT, MAXC+1]; device fills the last column
            # with the self-loop index: s = jt*128 + p - r0 if in range else -1
            idx_sb = cpool.tile([P, JT, NIDX], I16, tag="idx_sb")
            nc.sync.dma_start(
                out=idx_sb[:, :, 0:MAXC],
                in_=idx_in.ap().rearrange("j p m -> p j m"),
            )
            sval = cpool.tile([P, JT], I32, tag="sval")
            nc.gpsimd.iota(sval[:, :], pattern=[[P, JT]], base=0,
                           channel_multiplier=1)
            nc.vector.tensor_tensor(out=sval[:, :], in0=sval[:, :],
                                    in1=rb_sb[:, 0:1].to_broadcast([P, JT]),
                                    op=mybir.AluOpType.subtract)
            m1 = cpool.tile([P, JT], I32, tag="m1")
            nc.vector.tensor_scalar(out=m1[:, :], in0=sval[:, :], scalar1=0,
                                    scalar2=None, op0=mybir.AluOpType.is_ge)
            m2 = cpool.tile([P, JT], I32, tag="m2")
            nc.vector.tensor_scalar(out=m2[:, :], in0=sval[:, :],
                                    scalar1=NSHARD, scalar2=None,
                                    op0=mybir.AluOpType.is_lt)
            nc.vector.tensor_tensor(out=m1[:, :], in0=m1[:, :], in1=m2[:, :],
                                    op=mybir.AluOpType.mult)
            # sel = sval*m + (m-1): in-range -> sval, else -1
            nc.vector.tensor_tensor(out=sval[:, :], in0=sval[:, :],
                                    in1=m1[:, :], op=mybir.AluOpType.mult)
            nc.vector.tensor_scalar(out=m1[:, :], in0=m1[:, :], scalar1=1,
                                    scalar2=None, op0=mybir.AluOpType.subtract)
            nc.vector.tensor_tensor(out=sval[:, :], in0=sval[:, :],
                                    in1=m1[:, :], op=mybir.AluOpType.add)
            nc.vector.tensor_copy(out=idx_sb[:, :, MAXC], in_=sval[:, :])

            # ---------- build canvas slabs in SBUF via local_scatter ----------
            canv_sb = []
            for j in range(JT):
                cm = canvpool.tile([P, NSHARD], BF16, tag="cm")
                nc.gpsimd.local_scatter(
                    out_ap=cm[:, :],
                    data_ap=ones_data[:, :],
                    idxs_ap=idx_sb[:, j, :],
                    channels=P, num_elems=NSHARD, num_idxs=NIDX)
                canv_sb.append(cm)

            # ---------- support = x_c @ W (PE), fp32 ----------
            xT_sb = cpool.tile([P, NSHARD // P, DIN], BF16, tag="xT_sb")
            sup_sb = cpool.tile([P, NSHARD // P, DOUT], F32, tag="sup_sb")
            for t in range(NSHARD // P):
                ps_t = psA.tile([P, P], F32, tag="ps_t")
                nc.tensor.transpose(out=ps_t[:, :], in_=xc_sb[:, t, :],
                                    identity=ident[:, :])
                nc.vector.tensor_copy(out=xT_sb[:, t, :], in_=ps_t[:, :])
            for t in range(NSHARD // P):
                ps_s = psA.tile([P, P], F32, tag="ps_s")
                nc.tensor.matmul(out=ps_s[:, :], lhsT=xT_sb[:, t, :],
                                 rhs=w_bf[:, :], start=True, stop=True)
                nc.vector.tensor_copy(out=sup_sb[:, t, :], in_=ps_s[:, :])

            # ---------- degree via PE row-sum of the slabs ----------
            H = NSHARD // 2
            ps_d0 = psD.tile([1, H], F32, tag="ps_d0")
            ps_d1 = psD.tile([1, H], F32, tag="ps_d1")
            for j in range(JT):
                first = (j == 0)
                last = (j == JT - 1)
                nc.tensor.matmul(out=ps_d0[:, :], lhsT=ones_col[:, :],
                                 rhs=canv_sb[j][:, 0:H],
                                 start=first, stop=last)
                nc.tensor.matmul(out=ps_d1[:, :], lhsT=ones_col[:, :],
                                 rhs=canv_sb[j][:, H:NSHARD],
                                 start=first, stop=last)

            deg_sb = wpool.tile([1, NSHARD], F32, tag="deg_sb")
            nc.vector.tensor_copy(out=deg_sb[:, 0:H], in_=ps_d0[:, :])
            nc.vector.tensor_copy(out=deg_sb[:, H:NSHARD], in_=ps_d1[:, :])

            # redistribute deg [1, 1024] -> [128, 8] with node = t*128 + p
            nc.sync.dma_start(out=deg_dram.ap(), in_=deg_sb[0:1, :])
            degp = wpool.tile([P, NSHARD // P], F32, tag="degp")
            nc.sync.dma_start(
                out=degp[:, :],
                in_=deg_dram.ap().rearrange("(f p) one -> p (f one)", p=P),
            )
            dis = wpool.tile([P, NSHARD // P], F32, tag="dis")
            nc.vector.reciprocal(out=dis[:, :], in_=degp[:, :])
            nc.scalar.sqrt(out=dis[:, :], in_=dis[:, :])

            # ---------- z = dis * support, bounce + AllGather ----------
            z_sb = wpool.tile([P, NSHARD // P, DOUT], BF16, tag="z_sb")
            for t in range(NSHARD // P):
                nc.vector.tensor_scalar_mul(out=z_sb[:, t, :],
                                            in0=sup_sb[:, t, :],
                                            scalar1=dis[:, t:t + 1])
            nc.sync.dma_start(
                out=z_dram.ap().rearrange("(t p) d -> p t d", p=P),
                in_=z_sb[:, :, :])

            nc.gpsimd.collective_compute(
                "AllGather", mybir.AluOpType.bypass,
                replica_groups=[list(range(NCORES))],
                ins=[z_dram.ap().opt()], outs=[zall_dram.ap().opt()])

            zall_sb = wpool.tile([P, JT, DOUT], BF16, tag="zall_sb")
            nc.sync.dma_start(
                out=zall_sb[:, :, :],
                in_=zall_dram.ap().rearrange("(j p) d -> p j d", p=P))

            # ---------- main contraction out_T[d, i] ----------
            ps_o0 = psO.tile([P, H], F32, tag="ps_o0")
            ps_o1 = psO.tile([P, H], F32, tag="ps_o1")
            for j in range(JT):
                first = (j == 0)
                last = (j == JT - 1)
                nc.tensor.matmul(out=ps_o0[:, :], lhsT=zall_sb[:, j, :],
                                 rhs=canv_sb[j][:, 0:H],
                                 start=first, stop=last)
                nc.tensor.matmul(out=ps_o1[:, :], lhsT=zall_sb[:, j, :],
                                 rhs=canv_sb[j][:, H:NSHARD],
                                 start=first, stop=last)

            # row-side scale: out = dis_i * psum + bias
            disbig = wpool.tile([P, NSHARD], F32, tag="disbig")
            nc.sync.dma_start(
                out=disbig[:, :],
                in_=deg_dram.ap().rearrange("f one -> (one) f")
                .to_broadcast([P, NSHARD]))
            nc.vector.reciprocal(out=disbig[:, :], in_=disbig[:, :])
            nc.scalar.sqrt(out=disbig[:, :], in_=disbig[:, :])

            o_sb = wpool.tile([P, NSHARD], F32, tag="o_sb")
            nc.vector.tensor_tensor(out=o_sb[:, 0:H], in0=ps_o0[:, :],
                                    in1=disbig[:, 0:H],
                                    op=mybir.AluOpType.mult)
            nc.vector.tensor_tensor(out=o_sb[:, H:NSHARD], in0=ps_o1[:, :],
                                    in1=disbig[:, H:NSHARD],
                                    op=mybir.AluOpType.mult)
            nc.vector.tensor_scalar_add(out=o_sb[:, :], in0=o_sb[:, :],
                                        scalar1=bias_sb[:, 0:1])
            nc.sync.dma_start(out=out_t[:, :], in_=o_sb[:, :])

    nc.compile()
    return nc


def shard_inputs(x, weight, bias, edge_index):
    """Host-side sharding: row-partition nodes over cores; bucket each core's
    edges by destination column into fixed-size index lists."""
    x = np.asarray(x, dtype=np.float32)
    weight = np.ascontiguousarray(np.asarray(weight, dtype=np.float32))
    bias = np.asarray(bias, dtype=np.float32).reshape(DOUT, 1)
    ei = np.asarray(edge_index, dtype=np.int64)
    rows, cols = ei[0], ei[1]

    in_maps = []
    for c in range(NCORES):
        r0 = c * NSHARD
        m = (rows >= r0) & (rows < r0 + NSHARD) & (rows != cols)
        # unique (col, local_row) keys: sorted buckets, duplicates collapsed
        # (local_scatter forbids duplicate indices; values are all 1.0)
        key = np.unique(cols[m] * NSHARD + (rows[m] - r0))
        col = key // NSHARD
        lr = (key % NSHARD).astype(np.int16)
        cnt = np.bincount(col, minlength=N)
        if cnt.max() > MAXC:
            raise ValueError(f"core {c}: column bucket {cnt.max()} > {MAXC}")
        idx = np.full((N, MAXC), -1, dtype=np.int16)
        pos = np.arange(len(key)) - np.repeat(np.cumsum(cnt) - cnt, cnt)
        idx[col, pos] = lr
        in_maps.append({
            "x_c": np.ascontiguousarray(x[r0:r0 + NSHARD]),
            "w": weight,
            "bias_in": bias,
            "idx_in": idx.reshape(JT, P, MAXC),
            "rowbase": np.full((P, 1), r0, dtype=np.int32),
        })
    return in_maps


def _install_ntff_hook():
    """Provide antenv.axon_hooks if the image lacks it (profiling only)."""
    try:
        import antenv.axon_hooks  # noqa: F401
        return
    except ImportError:
        pass
    import types
    import antenv
    from trn_agent_boot.trn_boot import _ntff_profile_via_ctypes

    hook = _ntff_profile_via_ctypes("/opt/axon/libaxon_pjrt.so")
    mod = types.ModuleType("antenv.axon_hooks")
    mod._hook = hook
    mod.get_axon_ntff_profile_hook = lambda: mod._hook
    mod.set_axon_ntff_profile_hook = lambda h: setattr(mod, "_hook", h)
    sys.modules["antenv.axon_hooks"] = mod
    antenv.axon_hooks = mod


def kernel(x, weight, bias, edge_index, _trace=False):
    from concourse import bass_utils

    if _trace:
        _install_ntff_hook()

    if "nc" not in _COMPILED:
        _COMPILED["nc"] = build_nc()
    nc = _COMPILED["nc"]

    in_maps = shard_inputs(x, weight, bias, edge_index)
    res = bass_utils.run_bass_kernel_spmd(
        nc, in_maps, core_ids=list(range(NCORES)), trace=_trace)
    if _trace:
        _COMPILED["last_results"] = res

    out = np.empty((N, DOUT), dtype=np.float32)
    for c in range(NCORES):
        out[c * NSHARD:(c + 1) * NSHARD, :] = res.results[c]["out_t"].T
    return out


# revision 16
# speedup vs baseline: 1.3646x; 1.3646x over previous
"""GCN layer kernel for Trainium2, 8 NeuronCores.

out = D^-1/2 (A + I) D^-1/2 (x @ W) + bias   with A built dense from edge_index
(scatter-set semantics => duplicate edges collapse, matching the reference).

Sharding: 1D node/row partition over 8 cores (hardcoded). Each core builds the
transposed adjacency slab A_T[j, i] = A[r0+i, j] for its 1024 rows directly in
SBUF as 64 bf16 tiles via gpsimd local_scatter (per-column index lists, host-
bucketed; self loops appended on device as an extra index column), row-sums the
slab on the PE (ones-vector matmul) for its nodes' degrees, computes
z = deg^-1/2 * (x_shard @ W), all-gathers z, then contracts
out_T[d, i] = sum_j z[j, d] * A_T[j, i] on the PE with fp32 PSUM accumulation,
scales rows by deg^-1/2 and adds bias. Host only shards/buckets inputs and
transposes/concats the outputs.
"""

import sys

for _p in ("/opt/trn_rl_repo", "/root/.axon_site/_ro/trn_rl_repo"):
    if _p not in sys.path:
        sys.path.append(_p)

import numpy as np

import concourse.bacc as bacc
import concourse.bass as bass
import concourse.mybir as mybir
import concourse.tile as tile
from concourse.masks import make_identity

# Problem shape (hardcoded per contract)
N = 8192
DIN = 128
DOUT = 128
P = 128
NCORES = 8
NSHARD = N // NCORES          # 1024 rows per core
JT = N // P                   # 64 contraction tiles
MAXC = 23                     # max bucketed edges per (core, column)
NIDX = MAXC + 1               # + device-appended self-loop column (even)

BF16 = mybir.dt.bfloat16
F32 = mybir.dt.float32
I32 = mybir.dt.int32
I16 = mybir.dt.int16

_COMPILED = {}


def build_nc(debug: bool = False, dbg_taps: bool = False):
    nc = bacc.Bacc("TRN2", target_bir_lowering=False, debug=debug,
                   enable_asserts=False, num_devices=NCORES)

    # I/O
    x_c = nc.dram_tensor("x_c", [NSHARD, DIN], F32, kind="ExternalInput")
    w = nc.dram_tensor("w", [DIN, DOUT], F32, kind="ExternalInput")
    bias_in = nc.dram_tensor("bias_in", [DOUT, 1], F32, kind="ExternalInput")
    idx_in = nc.dram_tensor("idx_in", [JT, P, MAXC], I16, kind="ExternalInput")
    rowbase = nc.dram_tensor("rowbase", [P, 1], I32, kind="ExternalInput")
    out_t = nc.dram_tensor("out_t", [DOUT, NSHARD], F32, kind="ExternalOutput")
    dbg = {}
    if dbg_taps:
        dbg["canvas_out"] = nc.dram_tensor(
            "canvas_out", [JT, P, NSHARD], BF16, kind="ExternalOutput")
        dbg["deg_out"] = nc.dram_tensor(
            "deg_out", [NSHARD, 1], F32, kind="ExternalOutput")
        dbg["zall_out"] = nc.dram_tensor(
            "zall_out", [N, DOUT], BF16, kind="ExternalOutput")

    # Internal DRAM
    z_dram = nc.dram_tensor("z_dram", [NSHARD, DOUT], BF16)
    deg_dram = nc.dram_tensor("deg_dram", [NSHARD, 1], F32)
    zall_dram = nc.dram_tensor("zall_dram", [N, DOUT], BF16, addr_space="Shared")

    with tile.TileContext(nc) as tc:
        with (
            tc.tile_pool(name="const", bufs=1) as cpool,
            tc.tile_pool(name="canv", bufs=JT) as canvpool,
            tc.tile_pool(name="work", bufs=1) as wpool,
            tc.tile_pool(name="psA", bufs=2, space="PSUM") as psA,
            tc.tile_pool(name="psD", bufs=1, space="PSUM") as psD,
            tc.tile_pool(name="psO", bufs=1, space="PSUM") as psO,
        ):
            # ---------- constants / small loads ----------
            ones_data = cpool.tile([P, NIDX], BF16, tag="ones_data")
            nc.gpsimd.memset(ones_data[:, :], 1.0)
            ones_col = cpool.tile([P, 1], BF16, tag="ones_col")
            nc.gpsimd.memset(ones_col[:, :], 1.0)

            ident = cpool.tile([P, P], F32, tag="ident")
            make_identity(nc, ident[:, :])

            w_sb = cpool.tile([DIN, DOUT], F32, tag="w_sb")
            nc.sync.dma_start(out=w_sb[:, :], in_=w[:, :])
            w_bf = cpool.tile([DIN, DOUT], BF16, tag="w_bf")
            nc.vector.tensor_copy(out=w_bf[:, :], in_=w_sb[:, :])
            bias_sb = cpool.tile([DOUT, 1], F32, tag="bias_sb")
            nc.sync.dma_start(out=bias_sb[:, :], in_=bias_in[:, :])

            rb_sb = cpool.tile([P, 1], I32, tag="rb_sb")
            nc.sync.dma_start(out=rb_sb[:, :], in_=rowbase[:, :])

            xc_sb = cpool.tile([P, NSHARD // P, DIN], F32, tag="xc_sb")
            nc.sync.dma_start(
                out=xc_sb[:, :, :],
                in_=x_c.ap().rearrange("(t p) d -> p t d", p=P),
            )

            # edge index lists [128, JT, MAXC+1]; device fills the last column
            # with the self-loop index: s = jt*128 + p - r0 if in range else -1
            idx_sb = cpool.tile([P, JT, NIDX], I16, tag="idx_sb")
            nc.sync.dma_start(
                out=idx_sb[:, :, 0:MAXC],
                in_=idx_in.ap().rearrange("j p m -> p j m"),
            )
            sval = cpool.tile([P, JT], I32, tag="sval")
            nc.gpsimd.iota(sval[:, :], pattern=[[P, JT]], base=0,
                           channel_multiplier=1)
            nc.vector.tensor_tensor(out=sval[:, :], in0=sval[:, :],
                                    in1=rb_sb[:, 0:1].to_broadcast([P, JT]),
                                    op=mybir.AluOpType.subtract)
            m1 = cpool.tile([P, JT], I32, tag="m1")
            nc.vector.tensor_scalar(out=m1[:, :], in0=sval[:, :], scalar1=0,
                                    scalar2=None, op0=mybir.AluOpType.is_ge)
            m2 = cpool.tile([P, JT], I32, tag="m2")
            nc.vector.tensor_scalar(out=m2[:, :], in0=sval[:, :],
                                    scalar1=NSHARD, scalar2=None,
                                    op0=mybir.AluOpType.is_lt)
            nc.vector.tensor_tensor(out=m1[:, :], in0=m1[:, :], in1=m2[:, :],
                                    op=mybir.AluOpType.mult)
            # sel = sval*m + (m-1): in-range -> sval, else -1
            nc.vector.tensor_tensor(out=sval[:, :], in0=sval[:, :],
                                    in1=m1[:, :], op=mybir.AluOpType.mult)
            nc.vector.tensor_scalar(out=m1[:, :], in0=m1[:, :], scalar1=1,
                                    scalar2=None, op0=mybir.AluOpType.subtract)
            nc.vector.tensor_tensor(out=sval[:, :], in0=sval[:, :],
                                    in1=m1[:, :], op=mybir.AluOpType.add)
            # write the self column from gpsimd: same-engine FIFO with the
            # local_scatter reads below (a DVE write here raced on HW)
            selfcol = nc.gpsimd.tensor_copy(out=idx_sb[:, :, MAXC],
                                            in_=sval[:, :])

            # ---------- build canvas slabs in SBUF via local_scatter ----------
            canv_sb = []
            for j in range(JT):
                cm = canvpool.tile([P, NSHARD], BF16, tag="cm")
                sc = nc.gpsimd.local_scatter(
                    out_ap=cm[:, :],
                    data_ap=ones_data[:, :],
                    idxs_ap=idx_sb[:, j, :],
                    channels=P, num_elems=NSHARD, num_idxs=NIDX)
                tile.add_dep_helper(sc.ins, selfcol.ins,
                                    reason="scatter after self-col write")
                canv_sb.append(cm)

            # ---------- support = x_c @ W (PE), fp32 ----------
            xT_sb = cpool.tile([P, NSHARD // P, DIN], BF16, tag="xT_sb")
            sup_sb = cpool.tile([P, NSHARD // P, DOUT], F32, tag="sup_sb")
            for t in range(NSHARD // P):
                ps_t = psA.tile([P, P], F32, tag="ps_t")
                nc.tensor.transpose(out=ps_t[:, :], in_=xc_sb[:, t, :],
                                    identity=ident[:, :])
                nc.vector.tensor_copy(out=xT_sb[:, t, :], in_=ps_t[:, :])
            for t in range(NSHARD // P):
                ps_s = psA.tile([P, P], F32, tag="ps_s")
                nc.tensor.matmul(out=ps_s[:, :], lhsT=xT_sb[:, t, :],
                                 rhs=w_bf[:, :], start=True, stop=True)
                nc.vector.tensor_copy(out=sup_sb[:, t, :], in_=ps_s[:, :])

            # ---------- degree via PE row-sum of the slabs ----------
            H = NSHARD // 2
            ps_d0 = psD.tile([1, H], F32, tag="ps_d0")
            ps_d1 = psD.tile([1, H], F32, tag="ps_d1")
            for j in range(JT):
                first = (j == 0)
                last = (j == JT - 1)
                nc.tensor.matmul(out=ps_d0[:, :], lhsT=ones_col[:, :],
                                 rhs=canv_sb[j][:, 0:H],
                                 start=first, stop=last)
                nc.tensor.matmul(out=ps_d1[:, :], lhsT=ones_col[:, :],
                                 rhs=canv_sb[j][:, H:NSHARD],
                                 start=first, stop=last)

            deg_sb = wpool.tile([1, NSHARD], F32, tag="deg_sb")
            nc.vector.tensor_copy(out=deg_sb[:, 0:H], in_=ps_d0[:, :])
            nc.vector.tensor_copy(out=deg_sb[:, H:NSHARD], in_=ps_d1[:, :])

            # redistribute deg [1, 1024] -> [128, 8] with node = t*128 + p
            nc.sync.dma_start(out=deg_dram.ap(), in_=deg_sb[0:1, :])
            degp = wpool.tile([P, NSHARD // P], F32, tag="degp")
            nc.sync.dma_start(
                out=degp[:, :],
                in_=deg_dram.ap().rearrange("(f p) one -> p (f one)", p=P),
            )
            dis = wpool.tile([P, NSHARD // P], F32, tag="dis")
            nc.vector.reciprocal(out=dis[:, :], in_=degp[:, :])
            nc.scalar.sqrt(out=dis[:, :], in_=dis[:, :])

            # ---------- z = dis * support, bounce + AllGather ----------
            z_sb = wpool.tile([P, NSHARD // P, DOUT], BF16, tag="z_sb")
            for t in range(NSHARD // P):
                nc.vector.tensor_scalar_mul(out=z_sb[:, t, :],
                                            in0=sup_sb[:, t, :],
                                            scalar1=dis[:, t:t + 1])
            nc.sync.dma_start(
                out=z_dram.ap().rearrange("(t p) d -> p t d", p=P),
                in_=z_sb[:, :, :])

            nc.gpsimd.collective_compute(
                "AllGather", mybir.AluOpType.bypass,
                replica_groups=[list(range(NCORES))],
                ins=[z_dram.ap().opt()], outs=[zall_dram.ap().opt()])

            zall_sb = wpool.tile([P, JT, DOUT], BF16, tag="zall_sb")
            nc.sync.dma_start(
                out=zall_sb[:, :, :],
                in_=zall_dram.ap().rearrange("(j p) d -> p j d", p=P))

            # ---------- main contraction out_T[d, i] ----------
            ps_o0 = psO.tile([P, H], F32, tag="ps_o0")
            ps_o1 = psO.tile([P, H], F32, tag="ps_o1")
            for j in range(JT):
                first = (j == 0)
                last = (j == JT - 1)
                nc.tensor.matmul(out=ps_o0[:, :], lhsT=zall_sb[:, j, :],
                                 rhs=canv_sb[j][:, 0:H],
                                 start=first, stop=last)
                nc.tensor.matmul(out=ps_o1[:, :], lhsT=zall_sb[:, j, :],
                                 rhs=canv_sb[j][:, H:NSHARD],
                                 start=first, stop=last)

            # row-side scale: out = dis_i * psum + bias
            disbig = wpool.tile([P, NSHARD], F32, tag="disbig")
            nc.sync.dma_start(
                out=disbig[:, :],
                in_=deg_dram.ap().rearrange("f one -> (one) f")
                .to_broadcast([P, NSHARD]))
            nc.vector.reciprocal(out=disbig[:, :], in_=disbig[:, :])
            nc.scalar.sqrt(out=disbig[:, :], in_=disbig[:, :])

            o_sb = wpool.tile([P, NSHARD], F32, tag="o_sb")
            nc.vector.tensor_tensor(out=o_sb[:, 0:H], in0=ps_o0[:, :],
                                    in1=disbig[:, 0:H],
                                    op=mybir.AluOpType.mult)
            nc.vector.tensor_tensor(out=o_sb[:, H:NSHARD], in0=ps_o1[:, :],
                                    in1=disbig[:, H:NSHARD],
                                    op=mybir.AluOpType.mult)
            nc.vector.tensor_scalar_add(out=o_sb[:, :], in0=o_sb[:, :],
                                        scalar1=bias_sb[:, 0:1])
            nc.sync.dma_start(out=out_t[:, :], in_=o_sb[:, :])

            if dbg_taps:
                for j in range(JT):
                    nc.sync.dma_start(out=dbg["canvas_out"][j], in_=canv_sb[j][:, :])
                nc.sync.dma_start(out=dbg["deg_out"].ap(), in_=deg_sb[0:1, :])
                nc.sync.dma_start(
                    out=dbg["zall_out"].ap().rearrange("(j p) d -> p j d", p=P),
                    in_=zall_sb[:, :, :])

    nc.compile()
    return nc


def shard_inputs(x, weight, bias, edge_index):
    """Host-side sharding: row-partition nodes over cores; bucket each core's
    edges by destination column into fixed-size index lists."""
    x = np.asarray(x, dtype=np.float32)
    weight = np.ascontiguousarray(np.asarray(weight, dtype=np.float32))
    bias = np.asarray(bias, dtype=np.float32).reshape(DOUT, 1)
    ei = np.asarray(edge_index, dtype=np.int64)
    rows, cols = ei[0], ei[1]

    in_maps = []
    for c in range(NCORES):
        r0 = c * NSHARD
        m = (rows >= r0) & (rows < r0 + NSHARD) & (rows != cols)
        # unique (col, local_row) keys: sorted buckets, duplicates collapsed
        # (local_scatter forbids duplicate indices; values are all 1.0)
        key = np.unique(cols[m] * NSHARD + (rows[m] - r0))
        col = key // NSHARD
        lr = (key % NSHARD).astype(np.int16)
        cnt = np.bincount(col, minlength=N)
        if cnt.max() > MAXC:
            raise ValueError(f"core {c}: column bucket {cnt.max()} > {MAXC}")
        idx = np.full((N, MAXC), -1, dtype=np.int16)
        pos = np.arange(len(key)) - np.repeat(np.cumsum(cnt) - cnt, cnt)
        idx[col, pos] = lr
        in_maps.append({
            "x_c": np.ascontiguousarray(x[r0:r0 + NSHARD]),
            "w": weight,
            "bias_in": bias,
            "idx_in": idx.reshape(JT, P, MAXC),
            "rowbase": np.full((P, 1), r0, dtype=np.int32),
        })
    return in_maps


def _install_ntff_hook():
    """Provide antenv.axon_hooks if the image lacks it (profiling only)."""
    try:
        import antenv.axon_hooks  # noqa: F401
        return
    except ImportError:
        pass
    import types
    import antenv
    from trn_agent_boot.trn_boot import _ntff_profile_via_ctypes

    hook = _ntff_profile_via_ctypes("/opt/axon/libaxon_pjrt.so")
    mod = types.ModuleType("antenv.axon_hooks")
    mod._hook = hook
    mod.get_axon_ntff_profile_hook = lambda: mod._hook
    mod.set_axon_ntff_profile_hook = lambda h: setattr(mod, "_hook", h)
    sys.modules["antenv.axon_hooks"] = mod
    antenv.axon_hooks = mod


def kernel(x, weight, bias, edge_index, _trace=False):
    from concourse import bass_utils

    if _trace:
        _install_ntff_hook()

    if "nc" not in _COMPILED:
        _COMPILED["nc"] = build_nc()
    nc = _COMPILED["nc"]

    in_maps = shard_inputs(x, weight, bias, edge_index)
    res = bass_utils.run_bass_kernel_spmd(
        nc, in_maps, core_ids=list(range(NCORES)), trace=_trace)
    if _trace:
        _COMPILED["last_results"] = res

    out = np.empty((N, DOUT), dtype=np.float32)
    for c in range(NCORES):
        out[c * NSHARD:(c + 1) * NSHARD, :] = res.results[c]["out_t"].T
    return out


# revision 17
# speedup vs baseline: 1.4099x; 1.0332x over previous
"""GCN layer kernel for Trainium2, 8 NeuronCores.

out = D^-1/2 (A + I) D^-1/2 (x @ W) + bias   with A built dense from edge_index
(scatter-set semantics => duplicate edges collapse, matching the reference).

Sharding: 1D node/row partition over 8 cores (hardcoded). Each core builds the
transposed adjacency slab A_T[j, i] = A[r0+i, j] for its 1024 rows directly in
SBUF as 64 bf16 tiles via gpsimd local_scatter (per-column index lists, host-
bucketed; self loops appended on device as an extra index column), row-sums the
slab on the PE (ones-vector matmul) for its nodes' degrees, computes
z = deg^-1/2 * (x_shard @ W), all-gathers z, then contracts
out_T[d, i] = sum_j z[j, d] * A_T[j, i] on the PE with fp32 PSUM accumulation,
scales rows by deg^-1/2 and adds bias. Host only shards/buckets inputs and
transposes/concats the outputs.
"""

import sys

for _p in ("/opt/trn_rl_repo", "/root/.axon_site/_ro/trn_rl_repo"):
    if _p not in sys.path:
        sys.path.append(_p)

import numpy as np

import concourse.bacc as bacc
import concourse.bass as bass
import concourse.mybir as mybir
import concourse.tile as tile
from concourse.masks import make_identity

# Problem shape (hardcoded per contract)
N = 8192
DIN = 128
DOUT = 128
P = 128
NCORES = 8
NSHARD = N // NCORES          # 1024 rows per core
JT = N // P                   # 64 contraction tiles
MAXC = 23                     # max bucketed edges per (core, column)
NIDX = MAXC + 1               # + device-appended self-loop column (even)

BF16 = mybir.dt.bfloat16
F32 = mybir.dt.float32
I32 = mybir.dt.int32
I16 = mybir.dt.int16

_COMPILED = {}


def build_nc(debug: bool = False, dbg_taps: bool = False):
    nc = bacc.Bacc("TRN2", target_bir_lowering=False, debug=debug,
                   enable_asserts=False, num_devices=NCORES)

    # I/O
    x_c = nc.dram_tensor("x_c", [NSHARD, DIN], F32, kind="ExternalInput")
    w = nc.dram_tensor("w", [DIN, DOUT], F32, kind="ExternalInput")
    bias_in = nc.dram_tensor("bias_in", [DOUT, 1], F32, kind="ExternalInput")
    idx_in = nc.dram_tensor("idx_in", [JT, P, NIDX], I16, kind="ExternalInput")
    rowbase = nc.dram_tensor("rowbase", [P, 1], I32, kind="ExternalInput")
    out_t = nc.dram_tensor("out_t", [DOUT, NSHARD], F32, kind="ExternalOutput")
    dbg = {}
    if dbg_taps:
        dbg["canvas_out"] = nc.dram_tensor(
            "canvas_out", [JT, P, NSHARD], BF16, kind="ExternalOutput")
        dbg["deg_out"] = nc.dram_tensor(
            "deg_out", [NSHARD, 1], F32, kind="ExternalOutput")
        dbg["zall_out"] = nc.dram_tensor(
            "zall_out", [N, DOUT], BF16, kind="ExternalOutput")

    # Internal DRAM
    z_dram = nc.dram_tensor("z_dram", [NSHARD, DOUT], BF16)
    deg_dram = nc.dram_tensor("deg_dram", [NSHARD, 1], F32)
    zall_dram = nc.dram_tensor("zall_dram", [N, DOUT], BF16, addr_space="Shared")

    with tile.TileContext(nc) as tc:
        with (
            tc.tile_pool(name="const", bufs=1) as cpool,
            tc.tile_pool(name="canv", bufs=JT) as canvpool,
            tc.tile_pool(name="work", bufs=1) as wpool,
            tc.tile_pool(name="psA", bufs=2, space="PSUM") as psA,
            tc.tile_pool(name="psD", bufs=1, space="PSUM") as psD,
            tc.tile_pool(name="psO", bufs=1, space="PSUM") as psO,
        ):
            # ---------- constants / small loads ----------
            ones_data = cpool.tile([P, NIDX], BF16, tag="ones_data")
            nc.gpsimd.memset(ones_data[:, :], 1.0)
            ones_col = cpool.tile([P, 1], BF16, tag="ones_col")
            nc.gpsimd.memset(ones_col[:, :], 1.0)

            ident = cpool.tile([P, P], F32, tag="ident")
            make_identity(nc, ident[:, :])

            w_sb = cpool.tile([DIN, DOUT], F32, tag="w_sb")
            nc.sync.dma_start(out=w_sb[:, :], in_=w[:, :])
            w_bf = cpool.tile([DIN, DOUT], BF16, tag="w_bf")
            nc.vector.tensor_copy(out=w_bf[:, :], in_=w_sb[:, :])
            bias_sb = cpool.tile([DOUT, 1], F32, tag="bias_sb")
            nc.sync.dma_start(out=bias_sb[:, :], in_=bias_in[:, :])

            xc_sb = cpool.tile([P, NSHARD // P, DIN], F32, tag="xc_sb")
            nc.sync.dma_start(
                out=xc_sb[:, :, :],
                in_=x_c.ap().rearrange("(t p) d -> p t d", p=P),
            )

            # edge index lists [128, JT, NIDX] (last column = self loops,
            # part of the host-side shard layout)
            idx_sb = cpool.tile([P, JT, NIDX], I16, tag="idx_sb")
            nc.sync.dma_start(
                out=idx_sb[:, :, :],
                in_=idx_in.ap().rearrange("j p m -> p j m"),
            )

            # ---------- build canvas slabs in SBUF via local_scatter ----------
            canv_sb = []
            for j in range(JT):
                cm = canvpool.tile([P, NSHARD], BF16, tag="cm")
                nc.gpsimd.local_scatter(
                    out_ap=cm[:, :],
                    data_ap=ones_data[:, :],
                    idxs_ap=idx_sb[:, j, :],
                    channels=P, num_elems=NSHARD, num_idxs=NIDX)
                canv_sb.append(cm)

            # ---------- support = x_c @ W (PE), fp32 ----------
            xT_sb = cpool.tile([P, NSHARD // P, DIN], BF16, tag="xT_sb")
            sup_sb = cpool.tile([P, NSHARD // P, DOUT], F32, tag="sup_sb")
            for t in range(NSHARD // P):
                ps_t = psA.tile([P, P], F32, tag="ps_t")
                nc.tensor.transpose(out=ps_t[:, :], in_=xc_sb[:, t, :],
                                    identity=ident[:, :])
                nc.vector.tensor_copy(out=xT_sb[:, t, :], in_=ps_t[:, :])
            for t in range(NSHARD // P):
                ps_s = psA.tile([P, P], F32, tag="ps_s")
                nc.tensor.matmul(out=ps_s[:, :], lhsT=xT_sb[:, t, :],
                                 rhs=w_bf[:, :], start=True, stop=True)
                nc.vector.tensor_copy(out=sup_sb[:, t, :], in_=ps_s[:, :])

            # ---------- degree via PE row-sum of the slabs ----------
            H = NSHARD // 2
            ps_d0 = psD.tile([1, H], F32, tag="ps_d0")
            ps_d1 = psD.tile([1, H], F32, tag="ps_d1")
            for j in range(JT):
                first = (j == 0)
                last = (j == JT - 1)
                nc.tensor.matmul(out=ps_d0[:, :], lhsT=ones_col[:, :],
                                 rhs=canv_sb[j][:, 0:H],
                                 start=first, stop=last)
                nc.tensor.matmul(out=ps_d1[:, :], lhsT=ones_col[:, :],
                                 rhs=canv_sb[j][:, H:NSHARD],
                                 start=first, stop=last)

            deg_sb = wpool.tile([1, NSHARD], F32, tag="deg_sb")
            nc.vector.tensor_copy(out=deg_sb[:, 0:H], in_=ps_d0[:, :])
            nc.vector.tensor_copy(out=deg_sb[:, H:NSHARD], in_=ps_d1[:, :])

            # redistribute deg [1, 1024] -> [128, 8] with node = t*128 + p
            nc.sync.dma_start(out=deg_dram.ap(), in_=deg_sb[0:1, :])
            degp = wpool.tile([P, NSHARD // P], F32, tag="degp")
            nc.sync.dma_start(
                out=degp[:, :],
                in_=deg_dram.ap().rearrange("(f p) one -> p (f one)", p=P),
            )
            dis = wpool.tile([P, NSHARD // P], F32, tag="dis")
            nc.vector.reciprocal(out=dis[:, :], in_=degp[:, :])
            nc.scalar.sqrt(out=dis[:, :], in_=dis[:, :])

            # ---------- z = dis * support, bounce + AllGather ----------
            z_sb = wpool.tile([P, NSHARD // P, DOUT], BF16, tag="z_sb")
            for t in range(NSHARD // P):
                nc.vector.tensor_scalar_mul(out=z_sb[:, t, :],
                                            in0=sup_sb[:, t, :],
                                            scalar1=dis[:, t:t + 1])
            nc.sync.dma_start(
                out=z_dram.ap().rearrange("(t p) d -> p t d", p=P),
                in_=z_sb[:, :, :])

            nc.gpsimd.collective_compute(
                "AllGather", mybir.AluOpType.bypass,
                replica_groups=[list(range(NCORES))],
                ins=[z_dram.ap().opt()], outs=[zall_dram.ap().opt()])

            zall_sb = wpool.tile([P, JT, DOUT], BF16, tag="zall_sb")
            nc.sync.dma_start(
                out=zall_sb[:, :, :],
                in_=zall_dram.ap().rearrange("(j p) d -> p j d", p=P))

            # ---------- main contraction out_T[d, i] ----------
            ps_o0 = psO.tile([P, H], F32, tag="ps_o0")
            ps_o1 = psO.tile([P, H], F32, tag="ps_o1")
            for j in range(JT):
                first = (j == 0)
                last = (j == JT - 1)
                nc.tensor.matmul(out=ps_o0[:, :], lhsT=zall_sb[:, j, :],
                                 rhs=canv_sb[j][:, 0:H],
                                 start=first, stop=last)
                nc.tensor.matmul(out=ps_o1[:, :], lhsT=zall_sb[:, j, :],
                                 rhs=canv_sb[j][:, H:NSHARD],
                                 start=first, stop=last)

            # row-side scale: out = dis_i * psum + bias
            disbig = wpool.tile([P, NSHARD], F32, tag="disbig")
            nc.sync.dma_start(
                out=disbig[:, :],
                in_=deg_dram.ap().rearrange("f one -> (one) f")
                .to_broadcast([P, NSHARD]))
            nc.vector.reciprocal(out=disbig[:, :], in_=disbig[:, :])
            nc.scalar.sqrt(out=disbig[:, :], in_=disbig[:, :])

            o_sb = wpool.tile([P, NSHARD], F32, tag="o_sb")
            nc.vector.tensor_tensor(out=o_sb[:, 0:H], in0=ps_o0[:, :],
                                    in1=disbig[:, 0:H],
                                    op=mybir.AluOpType.mult)
            nc.vector.tensor_tensor(out=o_sb[:, H:NSHARD], in0=ps_o1[:, :],
                                    in1=disbig[:, H:NSHARD],
                                    op=mybir.AluOpType.mult)
            nc.vector.tensor_scalar_add(out=o_sb[:, :], in0=o_sb[:, :],
                                        scalar1=bias_sb[:, 0:1])
            nc.sync.dma_start(out=out_t[:, :], in_=o_sb[:, :])

            if dbg_taps:
                for j in range(JT):
                    nc.sync.dma_start(out=dbg["canvas_out"][j], in_=canv_sb[j][:, :])
                nc.sync.dma_start(out=dbg["deg_out"].ap(), in_=deg_sb[0:1, :])
                nc.sync.dma_start(
                    out=dbg["zall_out"].ap().rearrange("(j p) d -> p j d", p=P),
                    in_=zall_sb[:, :, :])

    nc.compile()
    return nc


def shard_inputs(x, weight, bias, edge_index):
    """Host-side sharding: row-partition nodes over cores; bucket each core's
    edges by destination column into fixed-size index lists."""
    x = np.asarray(x, dtype=np.float32)
    weight = np.ascontiguousarray(np.asarray(weight, dtype=np.float32))
    bias = np.asarray(bias, dtype=np.float32).reshape(DOUT, 1)
    ei = np.asarray(edge_index, dtype=np.int64)
    rows, cols = ei[0], ei[1]

    in_maps = []
    for c in range(NCORES):
        r0 = c * NSHARD
        m = (rows >= r0) & (rows < r0 + NSHARD) & (rows != cols)
        # unique (col, local_row) keys: sorted buckets, duplicates collapsed
        # (local_scatter forbids duplicate indices; values are all 1.0)
        key = np.unique(cols[m] * NSHARD + (rows[m] - r0))
        col = key // NSHARD
        lr = (key % NSHARD).astype(np.int16)
        cnt = np.bincount(col, minlength=N)
        if cnt.max() > MAXC:
            raise ValueError(f"core {c}: column bucket {cnt.max()} > {MAXC}")
        idx = np.full((N, NIDX), -1, dtype=np.int16)
        pos = np.arange(len(key)) - np.repeat(np.cumsum(cnt) - cnt, cnt)
        idx[col, pos] = lr
        # self-loop column: diagonal entry for this core's own node range
        own = np.arange(r0, r0 + NSHARD)
        idx[own, MAXC] = (own - r0).astype(np.int16)
        in_maps.append({
            "x_c": np.ascontiguousarray(x[r0:r0 + NSHARD]),
            "w": weight,
            "bias_in": bias,
            "idx_in": idx.reshape(JT, P, NIDX),
            "rowbase": np.full((P, 1), r0, dtype=np.int32),
        })
    return in_maps


def _install_ntff_hook():
    """Provide antenv.axon_hooks if the image lacks it (profiling only)."""
    try:
        import antenv.axon_hooks  # noqa: F401
        return
    except ImportError:
        pass
    import types
    import antenv
    from trn_agent_boot.trn_boot import _ntff_profile_via_ctypes

    hook = _ntff_profile_via_ctypes("/opt/axon/libaxon_pjrt.so")
    mod = types.ModuleType("antenv.axon_hooks")
    mod._hook = hook
    mod.get_axon_ntff_profile_hook = lambda: mod._hook
    mod.set_axon_ntff_profile_hook = lambda h: setattr(mod, "_hook", h)
    sys.modules["antenv.axon_hooks"] = mod
    antenv.axon_hooks = mod


def kernel(x, weight, bias, edge_index, _trace=False):
    from concourse import bass_utils

    if _trace:
        _install_ntff_hook()

    if "nc" not in _COMPILED:
        _COMPILED["nc"] = build_nc()
    nc = _COMPILED["nc"]

    in_maps = shard_inputs(x, weight, bias, edge_index)
    res = bass_utils.run_bass_kernel_spmd(
        nc, in_maps, core_ids=list(range(NCORES)), trace=_trace)
    if _trace:
        _COMPILED["last_results"] = res

    out = np.empty((N, DOUT), dtype=np.float32)
    for c in range(NCORES):
        out[c * NSHARD:(c + 1) * NSHARD, :] = res.results[c]["out_t"].T
    return out


# revision 27
# speedup vs baseline: 1.7860x; 1.2668x over previous
"""GCN layer kernel for Trainium2, 8 NeuronCores.

out = D^-1/2 (A + I) D^-1/2 (x @ W) + bias   with A built dense from edge_index
(scatter-set semantics => duplicate edges collapse, matching the reference).

Sharding: 1D node/row partition over 8 cores (hardcoded). Each core builds the
transposed adjacency slab A_T[j, i] = A[r0+i, j] for its 1024 rows directly in
SBUF as 64 bf16 tiles via gpsimd local_scatter (per-column index lists bucketed
host-side as part of sharding; the diagonal self-loop column is shard-layout
metadata), row-sums the slab on the PE (ones-vector matmul) for its nodes'
degrees, and all-gathers the 4KB degree shards. Every core computes the full
support = x @ W on the PE (fp32) under the scatter window, scales it to
z = deg^-1/2 * support, then contracts out_T[d, i] = sum_j z[j, d] * A_T[j, i]
with fp32 PSUM accumulation, scales rows by its own deg^-1/2 and adds bias.
Host only shards/reorders inputs and transposes/concats the outputs.
"""

import sys

for _p in ("/opt/trn_rl_repo", "/root/.axon_site/_ro/trn_rl_repo"):
    if _p not in sys.path:
        sys.path.append(_p)

import numpy as np

import concourse.bacc as bacc
import concourse.bass as bass
import concourse.mybir as mybir
import concourse.tile as tile

# Problem shape (hardcoded per contract)
N = 8192
DIN = 128
DOUT = 128
P = 128
NCORES = 8
NSHARD = N // NCORES          # 1024 rows per core
JT = N // P                   # 64 contraction tiles
MAXC = 23                     # max bucketed edges per (core, column)
NIDX = MAXC + 1               # + self-loop column (even)

BF16 = mybir.dt.bfloat16
F32 = mybir.dt.float32
FP16 = mybir.dt.float16
I16 = mybir.dt.int16

_COMPILED = {}


def build_nc(debug: bool = False, dbg_taps: bool = False):
    nc = bacc.Bacc("TRN2", target_bir_lowering=False, debug=debug,
                   enable_asserts=False, num_devices=NCORES)

    # I/O (xt_in = x pre-transposed per 128-node tile: [jt, din, node])
    xt_in = nc.dram_tensor("xt_in", [JT, DIN, P], F32, kind="ExternalInput")
    w = nc.dram_tensor("w", [DIN, DOUT], F32, kind="ExternalInput")
    bias_in = nc.dram_tensor("bias_in", [DOUT, 1], F32, kind="ExternalInput")
    idx_in = nc.dram_tensor("idx_in", [P, JT, NIDX], I16, kind="ExternalInput")
    out_t = nc.dram_tensor("out_t", [DOUT, NSHARD], F32, kind="ExternalOutput")
    dbg = {}
    if dbg_taps:
        dbg["canvas_out"] = nc.dram_tensor(
            "canvas_out", [JT, P, NSHARD], BF16, kind="ExternalOutput")
        dbg["deg_out"] = nc.dram_tensor(
            "deg_out", [NSHARD, 1], F32, kind="ExternalOutput")
        dbg["zall_out"] = nc.dram_tensor(
            "zall_out", [N, DOUT], BF16, kind="ExternalOutput")

    # Internal DRAM
    deg_dram = nc.dram_tensor("deg_dram", [NSHARD, 1], F32)
    deg_dram_k = nc.dram_tensor("deg_dram_k", [NSHARD, 1], F32)
    degall_dram = nc.dram_tensor("degall_dram", [N, 1], F32,
                                 addr_space="Shared")

    with tile.TileContext(nc) as tc:
        with (
            tc.tile_pool(name="const", bufs=1) as cpool,
            tc.tile_pool(name="canv", bufs=JT) as canvpool,
            tc.tile_pool(name="xtp", bufs=8) as xtp,
            tc.tile_pool(name="work", bufs=1) as wpool,
            tc.tile_pool(name="psA", bufs=2, space="PSUM") as psA,
            tc.tile_pool(name="psD", bufs=1, space="PSUM") as psD,
            tc.tile_pool(name="psO", bufs=1, space="PSUM") as psO,
        ):
            # ---------- constants / small loads ----------
            ones_data = cpool.tile([P, NIDX], BF16, tag="ones_data")
            nc.gpsimd.memset(ones_data[:, :], 1.0)
            ones_col = cpool.tile([P, 1], BF16, tag="ones_col")
            nc.gpsimd.memset(ones_col[:, :], 1.0)

            # edge/self index lists, contiguous per partition: critical path
            idx_sb = cpool.tile([P, JT, NIDX], I16, tag="idx_sb")
            nc.sync.dma_start(out=idx_sb[:, :, :], in_=idx_in.ap())

            w_sb = cpool.tile([DIN, DOUT], F32, tag="w_sb")
            nc.sync.dma_start(out=w_sb[:, :], in_=w[:, :])
            bias_sb = cpool.tile([DOUT, 1], F32, tag="bias_sb")
            nc.sync.dma_start(out=bias_sb[:, :], in_=bias_in[:, :])

            # ---------- build canvas slabs in SBUF via local_scatter ----------
            canv_sb = []
            for j in range(JT):
                cm = canvpool.tile([P, NSHARD], BF16, tag="cm")
                nc.gpsimd.local_scatter(
                    out_ap=cm[:, :],
                    data_ap=ones_data[:, :],
                    idxs_ap=idx_sb[:, j, :],
                    channels=P, num_elems=NSHARD, num_idxs=NIDX)
                canv_sb.append(cm)

            # ---------- full support = x @ W (PE, fp32), under the scatters --
            sup_sb = cpool.tile([P, JT, DOUT], FP16, tag="sup_sb")
            for j in range(JT):
                xt = xtp.tile([DIN, P], F32, tag="xt")
                nc.scalar.dma_start(out=xt[:, :], in_=xt_in[j])
                ps_s = psA.tile([P, P], F32, tag="ps_s")
                nc.tensor.matmul(out=ps_s[:, :], lhsT=xt[:, :],
                                 rhs=w_sb[:, :], start=True, stop=True)
                nc.vector.tensor_copy(out=sup_sb[:, j, :], in_=ps_s[:, :])

            # ---------- degree via PE row-sum of the slabs ----------
            H = NSHARD // 2
            ps_d0 = psD.tile([1, H], F32, tag="ps_d0")
            ps_d1 = psD.tile([1, H], F32, tag="ps_d1")
            for j in range(JT):
                first = (j == 0)
                last = (j == JT - 1)
                nc.tensor.matmul(out=ps_d0[:, :], lhsT=ones_col[:, :],
                                 rhs=canv_sb[j][:, 0:H],
                                 start=first, stop=last)
                nc.tensor.matmul(out=ps_d1[:, :], lhsT=ones_col[:, :],
                                 rhs=canv_sb[j][:, H:NSHARD],
                                 start=first, stop=last)

            deg_sb = wpool.tile([1, NSHARD], F32, tag="deg_sb")
            nc.vector.tensor_copy(out=deg_sb[:, 0:H], in_=ps_d0[:, :])
            nc.vector.tensor_copy(out=deg_sb[:, H:NSHARD], in_=ps_d1[:, :])
            # permute to (p, jl) order so the post-AG load has 32B runs
            degperm = wpool.tile([1, NSHARD], F32, tag="degperm")
            nc.vector.tensor_copy(
                out=degperm[0:1, :].rearrange("one (p jl) -> one p jl", jl=8),
                in_=deg_sb[0:1, :].rearrange("one (jl p) -> one p jl", p=P))
            nc.sync.dma_start(out=deg_dram.ap(), in_=degperm[0:1, :])
            nc.sync.dma_start(out=deg_dram_k.ap(), in_=deg_sb[0:1, :])

            # ---------- AllGather the degree shards (4 KB) ----------
            nc.gpsimd.collective_compute(
                "AllGather", mybir.AluOpType.bypass,
                replica_groups=[list(range(NCORES))],
                ins=[deg_dram.ap().opt()], outs=[degall_dram.ap().opt()])

            # disall[p, j] = deg(node j*128+p) ^ -1/2
            # degall flat = c*1024 + p*8 + jl;  j = c*8 + jl
            disall = wpool.tile([P, JT], F32, tag="disall")
            nc.sync.dma_start(
                out=disall[:, :].rearrange("p (c jl) -> p c jl", c=NCORES),
                in_=degall_dram.ap().rearrange(
                    "(c p jl) one -> p c (jl one)", c=NCORES, p=P))
            nc.vector.reciprocal(out=disall[:, :], in_=disall[:, :])
            nc.scalar.sqrt(out=disall[:, :], in_=disall[:, :])

            # row-side scale factors for this core's rows (fills the AG window)
            disbig = wpool.tile([P, NSHARD], F32, tag="disbig")
            nc.sync.dma_start(
                out=disbig[:, :],
                in_=deg_dram_k.ap().rearrange("f one -> (one) f")
                .to_broadcast([P, NSHARD]))
            nc.vector.reciprocal(out=disbig[:, :], in_=disbig[:, :])
            nc.scalar.sqrt(out=disbig[:, :], in_=disbig[:, :])

            # ---------- z = disall * support (in place, fp16) ----------
            for j in range(JT):
                nc.vector.tensor_scalar_mul(out=sup_sb[:, j, :],
                                            in0=sup_sb[:, j, :],
                                            scalar1=disall[:, j:j + 1])

            # ---------- main contraction out_T[d, i] ----------
            ps_o0 = psO.tile([P, H], F32, tag="ps_o0")
            ps_o1 = psO.tile([P, H], F32, tag="ps_o1")
            for j in range(JT):
                first = (j == 0)
                last = (j == JT - 1)
                nc.tensor.matmul(out=ps_o0[:, :], lhsT=sup_sb[:, j, :],
                                 rhs=canv_sb[j][:, 0:H],
                                 start=first, stop=last)
                nc.tensor.matmul(out=ps_o1[:, :], lhsT=sup_sb[:, j, :],
                                 rhs=canv_sb[j][:, H:NSHARD],
                                 start=first, stop=last)

            o_sb = wpool.tile([P, NSHARD], F32, tag="o_sb")
            nc.vector.tensor_tensor(out=o_sb[:, 0:H], in0=ps_o0[:, :],
                                    in1=disbig[:, 0:H],
                                    op=mybir.AluOpType.mult)
            nc.vector.tensor_tensor(out=o_sb[:, H:NSHARD], in0=ps_o1[:, :],
                                    in1=disbig[:, H:NSHARD],
                                    op=mybir.AluOpType.mult)
            nc.vector.tensor_scalar_add(out=o_sb[:, :], in0=o_sb[:, :],
                                        scalar1=bias_sb[:, 0:1])
            nc.sync.dma_start(out=out_t[:, :], in_=o_sb[:, :])

            if dbg_taps:
                for j in range(JT):
                    nc.sync.dma_start(out=dbg["canvas_out"][j],
                                      in_=canv_sb[j][:, :])
                nc.sync.dma_start(out=dbg["deg_out"].ap(), in_=deg_sb[0:1, :])
                nc.sync.dma_start(
                    out=dbg["zall_out"].ap().rearrange("(j p) d -> p j d", p=P),
                    in_=sup_sb[:, :, :])

    nc.compile()
    return nc


def shard_inputs(x, weight, bias, edge_index):
    """Host-side sharding: row-partition nodes over cores; bucket each core's
    edges by destination column into fixed-size index lists (layout prep)."""
    x = np.asarray(x, dtype=np.float32)
    weight = np.ascontiguousarray(np.asarray(weight, dtype=np.float32))
    bias = np.asarray(bias, dtype=np.float32).reshape(DOUT, 1)
    ei = np.asarray(edge_index, dtype=np.int64)
    rows, cols = ei[0], ei[1]

    # x tiles pre-transposed to [jt, din, node] (replicated to every core)
    xt = np.ascontiguousarray(x.reshape(JT, P, DIN).transpose(0, 2, 1))

    in_maps = []
    for c in range(NCORES):
        r0 = c * NSHARD
        m = (rows >= r0) & (rows < r0 + NSHARD) & (rows != cols)
        # unique (col, local_row) keys: sorted buckets, duplicates collapsed
        # (local_scatter forbids duplicate indices; values are all 1.0)
        key = np.unique(cols[m] * NSHARD + (rows[m] - r0))
        col = key // NSHARD
        lr = (key % NSHARD).astype(np.int16)
        cnt = np.bincount(col, minlength=N)
        if cnt.max() > MAXC:
            raise ValueError(f"core {c}: column bucket {cnt.max()} > {MAXC}")
        idx = np.full((N, NIDX), -1, dtype=np.int16)
        pos = np.arange(len(key)) - np.repeat(np.cumsum(cnt) - cnt, cnt)
        idx[col, pos] = lr
        # self-loop column: diagonal entries for this core's own node range
        own = np.arange(r0, r0 + NSHARD)
        idx[own, MAXC] = (own - r0).astype(np.int16)
        # device layout [P, JT, NIDX]: partition p holds columns jt*128+p
        idx_dev = np.ascontiguousarray(
            idx.reshape(JT, P, NIDX).transpose(1, 0, 2))
        in_maps.append({
            "xt_in": xt,
            "w": weight,
            "bias_in": bias,
            "idx_in": idx_dev,
        })
    return in_maps


def _install_ntff_hook():
    """Provide antenv.axon_hooks if the image lacks it (profiling only)."""
    try:
        import antenv.axon_hooks  # noqa: F401
        return
    except ImportError:
        pass
    import types
    import antenv
    from trn_agent_boot.trn_boot import _ntff_profile_via_ctypes

    hook = _ntff_profile_via_ctypes("/opt/axon/libaxon_pjrt.so")
    mod = types.ModuleType("antenv.axon_hooks")
    mod._hook = hook
    mod.get_axon_ntff_profile_hook = lambda: mod._hook
    mod.set_axon_ntff_profile_hook = lambda h: setattr(mod, "_hook", h)
    sys.modules["antenv.axon_hooks"] = mod
    antenv.axon_hooks = mod


def kernel(x, weight, bias, edge_index, _trace=False):
    from concourse import bass_utils

    if _trace:
        _install_ntff_hook()

    if "nc" not in _COMPILED:
        _COMPILED["nc"] = build_nc()
    nc = _COMPILED["nc"]

    in_maps = shard_inputs(x, weight, bias, edge_index)
    res = bass_utils.run_bass_kernel_spmd(
        nc, in_maps, core_ids=list(range(NCORES)), trace=_trace)
    if _trace:
        _COMPILED["last_results"] = res

    out = np.empty((N, DOUT), dtype=np.float32)
    for c in range(NCORES):
        out[c * NSHARD:(c + 1) * NSHARD, :] = res.results[c]["out_t"].T
    return out


# revision 31
# speedup vs baseline: 1.8208x; 1.0195x over previous
"""GCN layer kernel for Trainium2, 8 NeuronCores.

out = D^-1/2 (A + I) D^-1/2 (x @ W) + bias   with A built dense from edge_index
(scatter-set semantics => duplicate edges collapse, matching the reference).

Sharding: 1D node/row partition over 8 cores (hardcoded). Each core builds the
transposed adjacency slab A_T[j, i] = A[r0+i, j] for its 1024 rows directly in
SBUF as 64 bf16 tiles via gpsimd local_scatter (per-column index lists bucketed
host-side as part of sharding; the diagonal self-loop column is shard-layout
metadata), row-sums the slab on the PE (ones-vector matmul) for its nodes'
degrees, and all-gathers the 4KB degree shards. Every core computes the full
support = x @ W on the PE (fp32) under the scatter window, scales it to
z = deg^-1/2 * support, then contracts out_T[d, i] = sum_j z[j, d] * A_T[j, i]
with fp32 PSUM accumulation, scales rows by its own deg^-1/2 and adds bias.
Host only shards/reorders inputs and transposes/concats the outputs.
"""

import sys

for _p in ("/opt/trn_rl_repo", "/root/.axon_site/_ro/trn_rl_repo"):
    if _p not in sys.path:
        sys.path.append(_p)

import numpy as np

import concourse.bacc as bacc
import concourse.bass as bass
import concourse.mybir as mybir
import concourse.tile as tile

# Problem shape (hardcoded per contract)
N = 8192
DIN = 128
DOUT = 128
P = 128
NCORES = 8
NSHARD = N // NCORES          # 1024 rows per core
JT = N // P                   # 64 contraction tiles
MAXC = 23                     # max bucketed edges per (core, column)
NIDX = MAXC + 1               # + self-loop column (even)

BF16 = mybir.dt.bfloat16
F32 = mybir.dt.float32
FP16 = mybir.dt.float16
I16 = mybir.dt.int16

_COMPILED = {}


def build_nc(debug: bool = False, dbg_taps: bool = False):
    nc = bacc.Bacc("TRN2", target_bir_lowering=False, debug=debug,
                   enable_asserts=False, num_devices=NCORES)

    # I/O (xt_in = x pre-transposed per 128-node tile: [jt, din, node])
    xt_in = nc.dram_tensor("xt_in", [JT, DIN, P], F32, kind="ExternalInput")
    w = nc.dram_tensor("w", [DIN, DOUT], F32, kind="ExternalInput")
    bias_in = nc.dram_tensor("bias_in", [DOUT, 1], F32, kind="ExternalInput")
    idx_in = nc.dram_tensor("idx_in", [P, JT, NIDX], I16, kind="ExternalInput")
    out_t = nc.dram_tensor("out_t", [DOUT, NSHARD], F32, kind="ExternalOutput")
    dbg = {}
    if dbg_taps:
        dbg["canvas_out"] = nc.dram_tensor(
            "canvas_out", [JT, P, NSHARD], BF16, kind="ExternalOutput")
        dbg["deg_out"] = nc.dram_tensor(
            "deg_out", [NSHARD, 1], F32, kind="ExternalOutput")
        dbg["zall_out"] = nc.dram_tensor(
            "zall_out", [N, DOUT], BF16, kind="ExternalOutput")

    # Internal DRAM
    deg_dram = nc.dram_tensor("deg_dram", [NSHARD], F32)
    deg_dram_k = nc.dram_tensor("deg_dram_k", [NSHARD, 1], F32)
    degall_dram = nc.dram_tensor("degall_dram", [N], F32,
                                 addr_space="Shared")

    with tile.TileContext(nc) as tc:
        with (
            tc.tile_pool(name="const", bufs=1) as cpool,
            tc.tile_pool(name="canv", bufs=JT) as canvpool,
            tc.tile_pool(name="xtp", bufs=8) as xtp,
            tc.tile_pool(name="work", bufs=1) as wpool,
            tc.tile_pool(name="psA", bufs=2, space="PSUM") as psA,
            tc.tile_pool(name="psD", bufs=1, space="PSUM") as psD,
            tc.tile_pool(name="psO", bufs=1, space="PSUM") as psO,
        ):
            # ---------- constants / small loads ----------
            ones_data = cpool.tile([P, NIDX], BF16, tag="ones_data")
            nc.gpsimd.memset(ones_data[:, :], 1.0)
            ones_col = cpool.tile([P, 1], BF16, tag="ones_col")
            nc.gpsimd.memset(ones_col[:, :], 1.0)

            # tiny dummy scatter: triggers the ext-isa library IRAM load
            # early so the first real scatter doesn't pay it
            warm_idx = cpool.tile([16, 2], I16, tag="warm_idx")
            nc.gpsimd.memset(warm_idx[:, :], -1)
            warm_dst = cpool.tile([16, 2], BF16, tag="warm_dst")
            warm_dat = cpool.tile([16, 2], BF16, tag="warm_dat")
            nc.gpsimd.memset(warm_dat[:, :], 0.0)
            nc.gpsimd.local_scatter(
                out_ap=warm_dst[:, :], data_ap=warm_dat[:, :],
                idxs_ap=warm_idx[:, :], channels=16, num_elems=2, num_idxs=2)

            # edge/self index lists, contiguous per partition: critical path
            idx_sb = cpool.tile([P, JT, NIDX], I16, tag="idx_sb")
            nc.sync.dma_start(out=idx_sb[:, :, :], in_=idx_in.ap())

            w_sb = cpool.tile([DIN, DOUT], F32, tag="w_sb")
            nc.sync.dma_start(out=w_sb[:, :], in_=w[:, :])
            bias_sb = cpool.tile([DOUT, 1], F32, tag="bias_sb")
            nc.sync.dma_start(out=bias_sb[:, :], in_=bias_in[:, :])

            # ---------- build canvas slabs in SBUF via local_scatter ----------
            canv_sb = []
            for j in range(JT):
                cm = canvpool.tile([P, NSHARD], BF16, tag="cm")
                nc.gpsimd.local_scatter(
                    out_ap=cm[:, :],
                    data_ap=ones_data[:, :],
                    idxs_ap=idx_sb[:, j, :],
                    channels=P, num_elems=NSHARD, num_idxs=NIDX)
                canv_sb.append(cm)

            # ---------- full support = x @ W (PE, fp32), under the scatters --
            sup_sb = cpool.tile([P, JT, DOUT], FP16, tag="sup_sb")
            for j in range(JT):
                xt = xtp.tile([DIN, P], F32, tag="xt")
                nc.scalar.dma_start(out=xt[:, :], in_=xt_in[j])
                ps_s = psA.tile([P, P], F32, tag="ps_s")
                nc.tensor.matmul(out=ps_s[:, :], lhsT=xt[:, :],
                                 rhs=w_sb[:, :], start=True, stop=True)
                nc.vector.tensor_copy(out=sup_sb[:, j, :], in_=ps_s[:, :])

            # ---------- degree via PE row-sum of the slabs ----------
            H = NSHARD // 2
            ps_d0 = psD.tile([1, H], F32, tag="ps_d0")
            ps_d1 = psD.tile([1, H], F32, tag="ps_d1")
            for j in range(JT):
                first = (j == 0)
                last = (j == JT - 1)
                nc.tensor.matmul(out=ps_d0[:, :], lhsT=ones_col[:, :],
                                 rhs=canv_sb[j][:, 0:H],
                                 start=first, stop=last)
                nc.tensor.matmul(out=ps_d1[:, :], lhsT=ones_col[:, :],
                                 rhs=canv_sb[j][:, H:NSHARD],
                                 start=first, stop=last)

            # deg shard in (p, jl) order (so the post-AG load has 32B runs),
            # permute fused into the PSUM->SBUF copies
            degperm = wpool.tile([1, NSHARD], F32, tag="degperm")
            nc.vector.tensor_copy(
                out=degperm[0:1, :].rearrange("one (p jl) -> one p jl", jl=8)
                [:, :, 0:4],
                in_=ps_d0[0:1, :].rearrange("one (jl p) -> one p jl", p=P))
            nc.vector.tensor_copy(
                out=degperm[0:1, :].rearrange("one (p jl) -> one p jl", jl=8)
                [:, :, 4:8],
                in_=ps_d1[0:1, :].rearrange("one (jl p) -> one p jl", p=P))
            nc.sync.dma_start(out=deg_dram.ap(), in_=degperm[0:1, :])
            # node-order copy for the row-side scale (off the AG path)
            deg_sb = wpool.tile([1, NSHARD], F32, tag="deg_sb")
            nc.vector.tensor_copy(out=deg_sb[:, 0:H], in_=ps_d0[:, :])
            nc.vector.tensor_copy(out=deg_sb[:, H:NSHARD], in_=ps_d1[:, :])
            nc.sync.dma_start(out=deg_dram_k.ap(), in_=deg_sb[0:1, :])

            # ---------- AllGather the degree shards (4 KB) ----------
            nc.gpsimd.collective_compute(
                "AllGather", mybir.AluOpType.bypass,
                replica_groups=[list(range(NCORES))],
                ins=[deg_dram.ap().opt()], outs=[degall_dram.ap().opt()])

            # disall[p, j] = deg(node j*128+p) ^ -1/2
            # degall flat = c*1024 + p*8 + jl;  j = c*8 + jl
            disall = wpool.tile([P, JT], F32, tag="disall")
            nc.sync.dma_start(
                out=disall[:, :].rearrange("p (c jl) -> p c jl", c=NCORES),
                in_=degall_dram.ap().rearrange(
                    "(c p jl) -> p c jl", c=NCORES, p=P))
            nc.vector.reciprocal(out=disall[:, :], in_=disall[:, :])
            nc.scalar.sqrt(out=disall[:, :], in_=disall[:, :])

            # row-side scale factors for this core's rows (fills the AG window)
            disbig = wpool.tile([P, NSHARD], F32, tag="disbig")
            nc.sync.dma_start(
                out=disbig[:, :],
                in_=deg_dram_k.ap().rearrange("f one -> (one) f")
                .to_broadcast([P, NSHARD]))
            nc.vector.reciprocal(out=disbig[:, :], in_=disbig[:, :])
            nc.scalar.sqrt(out=disbig[:, :], in_=disbig[:, :])

            # ---------- z = disall * support (in place, fp16) ----------
            for j in range(JT):
                nc.vector.tensor_scalar_mul(out=sup_sb[:, j, :],
                                            in0=sup_sb[:, j, :],
                                            scalar1=disall[:, j:j + 1])

            # ---------- main contraction out_T[d, i] ----------
            ps_o0 = psO.tile([P, H], F32, tag="ps_o0")
            ps_o1 = psO.tile([P, H], F32, tag="ps_o1")
            for j in range(JT):
                first = (j == 0)
                last = (j == JT - 1)
                nc.tensor.matmul(out=ps_o0[:, :], lhsT=sup_sb[:, j, :],
                                 rhs=canv_sb[j][:, 0:H],
                                 start=first, stop=last)
                nc.tensor.matmul(out=ps_o1[:, :], lhsT=sup_sb[:, j, :],
                                 rhs=canv_sb[j][:, H:NSHARD],
                                 start=first, stop=last)

            o_sb = wpool.tile([P, NSHARD], F32, tag="o_sb")
            nc.vector.tensor_tensor(out=o_sb[:, 0:H], in0=ps_o0[:, :],
                                    in1=disbig[:, 0:H],
                                    op=mybir.AluOpType.mult)
            nc.vector.tensor_tensor(out=o_sb[:, H:NSHARD], in0=ps_o1[:, :],
                                    in1=disbig[:, H:NSHARD],
                                    op=mybir.AluOpType.mult)
            nc.vector.tensor_scalar_add(out=o_sb[:, :], in0=o_sb[:, :],
                                        scalar1=bias_sb[:, 0:1])
            nc.sync.dma_start(out=out_t[:, :], in_=o_sb[:, :])

            if dbg_taps:
                for j in range(JT):
                    nc.sync.dma_start(out=dbg["canvas_out"][j],
                                      in_=canv_sb[j][:, :])
                nc.sync.dma_start(out=dbg["deg_out"].ap(), in_=deg_sb[0:1, :])
                nc.sync.dma_start(
                    out=dbg["zall_out"].ap().rearrange("(j p) d -> p j d", p=P),
                    in_=sup_sb[:, :, :])

    nc.compile()
    return nc


def shard_inputs(x, weight, bias, edge_index):
    """Host-side sharding: row-partition nodes over cores; bucket each core's
    edges by destination column into fixed-size index lists (layout prep)."""
    x = np.asarray(x, dtype=np.float32)
    weight = np.ascontiguousarray(np.asarray(weight, dtype=np.float32))
    bias = np.asarray(bias, dtype=np.float32).reshape(DOUT, 1)
    ei = np.asarray(edge_index, dtype=np.int64)
    rows, cols = ei[0], ei[1]

    # x tiles pre-transposed to [jt, din, node] (replicated to every core)
    xt = np.ascontiguousarray(x.reshape(JT, P, DIN).transpose(0, 2, 1))

    in_maps = []
    for c in range(NCORES):
        r0 = c * NSHARD
        m = (rows >= r0) & (rows < r0 + NSHARD) & (rows != cols)
        # unique (col, local_row) keys: sorted buckets, duplicates collapsed
        # (local_scatter forbids duplicate indices; values are all 1.0)
        key = np.unique(cols[m] * NSHARD + (rows[m] - r0))
        col = key // NSHARD
        lr = (key % NSHARD).astype(np.int16)
        cnt = np.bincount(col, minlength=N)
        if cnt.max() > MAXC:
            raise ValueError(f"core {c}: column bucket {cnt.max()} > {MAXC}")
        idx = np.full((N, NIDX), -1, dtype=np.int16)
        pos = np.arange(len(key)) - np.repeat(np.cumsum(cnt) - cnt, cnt)
        idx[col, pos] = lr
        # self-loop column: diagonal entries for this core's own node range
        own = np.arange(r0, r0 + NSHARD)
        idx[own, MAXC] = (own - r0).astype(np.int16)
        # device layout [P, JT, NIDX]: partition p holds columns jt*128+p
        idx_dev = np.ascontiguousarray(
            idx.reshape(JT, P, NIDX).transpose(1, 0, 2))
        in_maps.append({
            "xt_in": xt,
            "w": weight,
            "bias_in": bias,
            "idx_in": idx_dev,
        })
    return in_maps


def _install_ntff_hook():
    """Provide antenv.axon_hooks if the image lacks it (profiling only)."""
    try:
        import antenv.axon_hooks  # noqa: F401
        return
    except ImportError:
        pass
    import types
    import antenv
    from trn_agent_boot.trn_boot import _ntff_profile_via_ctypes

    hook = _ntff_profile_via_ctypes("/opt/axon/libaxon_pjrt.so")
    mod = types.ModuleType("antenv.axon_hooks")
    mod._hook = hook
    mod.get_axon_ntff_profile_hook = lambda: mod._hook
    mod.set_axon_ntff_profile_hook = lambda h: setattr(mod, "_hook", h)
    sys.modules["antenv.axon_hooks"] = mod
    antenv.axon_hooks = mod


def kernel(x, weight, bias, edge_index, _trace=False):
    from concourse import bass_utils

    if _trace:
        _install_ntff_hook()

    if "nc" not in _COMPILED:
        _COMPILED["nc"] = build_nc()
    nc = _COMPILED["nc"]

    in_maps = shard_inputs(x, weight, bias, edge_index)
    res = bass_utils.run_bass_kernel_spmd(
        nc, in_maps, core_ids=list(range(NCORES)), trace=_trace)
    if _trace:
        _COMPILED["last_results"] = res

    out = np.empty((N, DOUT), dtype=np.float32)
    for c in range(NCORES):
        out[c * NSHARD:(c + 1) * NSHARD, :] = res.results[c]["out_t"].T
    return out
